# revision 12
# baseline (speedup 1.0000x reference)
"""MeshGraphNet on 8 Trainium2 NeuronCores (Bass/Tile, SPMD).

Strategy
--------
- Nodes are ranked by scatter-degree (deg over edge_index[0]) and dealt
  round-robin to 8 cores, so every core holds NSH nodes with a near-identical
  degree profile. Edges live on the core that owns their `row` endpoint
  (the scatter-add destination), sorted by destination, with each node's
  edges padded to a common per-position window size D[i] (the max degree of
  the 8 nodes dealt at position i). That makes the program layout (window
  sizes, reduce regions) identical across cores: SPMD with per-core data.
- All activations feature-major in SBUF ([H=128 partitions, items free]).
  Edge features for the whole core-shard stay resident in SBUF in bf16.
- Gathers of x[row]/x[col] use the dma_gather(transpose=True) custom DMA
  against a bf16 x-table in DRAM, yielding feature-major tiles directly.
- scatter-add == windowed segment-reduce over the destination-sorted edge
  features (DVE tensor_reduce per equal-window region). Dummy-slot
  contributions are cancelled with a host-computed rank-1 correction
  (K=1 matmul) folded into the node MLP.
- Per layer, updated node features are AllGather'd into the DRAM x-table.
- fp32 is kept for: node residual stream, encoder/decoder first matmuls,
  PSUM accumulation, biases. Everything else bf16.
"""

import numpy as np
import ml_dtypes

import concourse.bacc as bacc
import concourse.mybir as mybir
import concourse.tile as tile
from concourse.bass_utils import run_bass_kernel_spmd
from concourse._compat import axon_active

BF16 = ml_dtypes.bfloat16
NCORES = 8
H = 128
T = 512          # matmul tile (free dim)
G = 4096         # gather chunk (indices per dma_gather)
P = 128

F32 = mybir.dt.float32
BF = mybir.dt.bfloat16
I16 = mybir.dt.int16


def _bf(a):
    return np.asarray(a, np.float32).astype(BF16)


class Cfg:
    def __init__(self, N, E, L, NI, EI, NO):
        self.N, self.E, self.L = N, E, L
        self.NI, self.EI, self.NO = NI, EI, NO
        assert N % NCORES == 0
        self.NSH = N // NCORES                      # nodes per core
        self.NSH_pad = -(-self.NSH // T) * T        # padded to tile multiple
        self.STRIDE = self.NSH_pad + 16             # per-core block in x-table
        self.NTOT = NCORES * self.STRIDE            # x-table rows
        self.ZERO_ROW = self.NSH_pad                # core-0 zero row (dummies)
        # filled by preprocess:
        self.E_pad = None
        self.NCH = None
        self.groups = None                          # [(slot_off, node_off, n, d)]


# ----------------------------------------------------------------------------
# Host preprocessing
# ----------------------------------------------------------------------------

def preprocess(inputs):
    x = np.asarray(inputs["x"], np.float32)
    edge_attr = np.asarray(inputs["edge_attr"], np.float32)
    ei = np.asarray(inputs["edge_index"]).astype(np.int64)
    row, col = ei[0], ei[1]

    N, NI = x.shape
    E, EI = edge_attr.shape
    L = np.asarray(inputs["edge_w1"]).shape[0]
    NO = np.asarray(inputs["nd_w2"]).shape[1]
    cfg = Cfg(N, E, L, NI, EI, NO)
    NSH, NSH_pad = cfg.NSH, cfg.NSH_pad

    deg = np.bincount(row, minlength=N)
    order = np.argsort(-deg, kind="stable")         # node ids, degree desc
    rank = np.empty(N, np.int64)
    rank[order] = np.arange(N)
    core_of = rank % NCORES
    pos_of = rank // NCORES
    permpos = core_of * cfg.STRIDE + pos_of         # x-table row per node

    D = np.maximum(deg[order[0::NCORES]], 1)        # common window sizes [NSH]
    Woff = np.concatenate([[0], np.cumsum(D)])
    S = int(Woff[-1])
    cfg.E_pad = -(-S // G) * G
    cfg.NCH = cfg.E_pad // G

    # reduce groups: runs of equal D
    groups = []
    i = 0
    while i < NSH:
        j = i
        while j < NSH and D[j] == D[i]:
            j += 1
        groups.append((int(Woff[i]), i, j - i, int(D[i])))
        i = j
    cfg.groups = groups

    # --- per-core edge layout ---
    edge_order = np.argsort(row, kind="stable")
    start = np.concatenate([[0], np.cumsum(deg)])   # edges of node n: edge_order[start[n]:start[n]+deg[n]]

    per_core = []
    for c in range(NCORES):
        nodes_c = order[NCORES * np.arange(NSH) + c]
        degs_c = deg[nodes_c]
        tot = int(degs_c.sum())
        cum0 = np.concatenate([[0], np.cumsum(degs_c)])[:-1]
        within = np.arange(tot) - np.repeat(cum0, degs_c)
        e_ids = edge_order[np.repeat(start[nodes_c], degs_c) + within]
        slots = np.repeat(Woff[:NSH], degs_c) + within

        col_idx = np.full(cfg.E_pad, cfg.ZERO_ROW, np.int32)
        row_idx = np.full(cfg.E_pad, cfg.ZERO_ROW, np.int32)
        attrT = np.zeros((EI, cfg.E_pad), np.float32)
        col_idx[slots] = permpos[col[e_ids]]
        row_idx[slots] = np.repeat(permpos[nodes_c], degs_c)
        attrT[:, slots] = edge_attr[e_ids].T

        ndum = np.zeros(NSH_pad, np.float32)
        ndum[:NSH] = D - degs_c

        xin = np.zeros((NI, NSH_pad), np.float32)
        xin[:, :NSH] = x[nodes_c].T

        per_core.append(dict(col_idx=col_idx, row_idx=row_idx, attrT=attrT,
                             ndum=ndum, xin=xin, nodes_c=nodes_c))

    return cfg, per_core


def _wrap_idx(idx):
    """[E_pad] int -> [128, E_pad//16] int16 (wrapped + replicated per Q7 core)."""
    n = idx.shape[0]
    assert n % 16 == 0
    w = np.asarray(idx, np.int16).reshape(n // 16, 16).T    # [16, n/16]
    return np.tile(w, (8, 1))


# ----------------------------------------------------------------------------
# Weight packing (shared between builder offsets and host fill)
# ----------------------------------------------------------------------------

class Pack:
    def __init__(self):
        self.cols = 0
        self.items = {}   # name -> (off, rows, ncols)

    def add(self, name, rows, ncols):
        self.items[name] = (self.cols, rows, ncols)
        self.cols += ncols

    def off(self, name):
        return self.items[name][0]


def make_packs(cfg):
    pb = Pack()           # bf16 pack [128, pb.cols]
    pb.add("ident", P, P)
    pb.add("zeros", 16, P)
    pb.add("ee_w1", cfg.EI, H)
    pb.add("ee_w2", H, H)
    pb.add("ne_w2", H, H)
    for l in range(cfg.L):
        pb.add(f"ew1a{l}", H, H)
        pb.add(f"ew1b{l}", H, H)
        pb.add(f"ew1c{l}", H, H)
        pb.add(f"ew2{l}", H, H)
        pb.add(f"nw1a{l}", H, H)
        pb.add(f"nw1b{l}", H, H)
        pb.add(f"nw2{l}", H, H)
        pb.add(f"sneg{l}", 1, H)
    pb.add("nd_w2", H, cfg.NO)
    pb.add("ndum", 1, cfg.NSH_pad)

    pf = Pack()           # f32 pack [128, pf.cols]
    pf.add("ne_w1", cfg.NI, H)
    pf.add("nd_w1", H, H)
    for nm in ["ne_b1", "ne_b2", "ee_b1", "ee_b2", "ndb1", "ndb2"]:
        pf.add(nm, H, 1)
    for l in range(cfg.L):
        for nm in ["eb1", "eb2", "nb1", "nb2"]:
            pf.add(f"{nm}{l}", H, 1)
    return pb, pf


def fill_packs(cfg, pb, pf, inputs, ndum):
    """Returns (wbf [128, pb.cols] bf16, wf32 [128, pf.cols] f32).
    Also computes the dummy-edge chain for sneg vectors."""
    W = {k: np.asarray(v, np.float32) for k, v in inputs.items()
         if k not in ("x", "edge_attr", "edge_index", "batch")}

    # dummy-edge value chain (mirrors device bf16 roundings)
    def mm(a, w):   # fp32 matmul of bf16-rounded operands
        return _bf(a).astype(np.float32) @ _bf(w).astype(np.float32)

    ed = _bf(np.maximum(mm(np.zeros(cfg.EI), W["ee_w1"]) + W["ee_b1"], 0))
    ed = _bf(mm(ed, W["ee_w2"]) + W["ee_b2"])                 # e0 for dummy slot
    sneg = []
    for l in range(cfg.L):
        h = _bf(np.maximum(mm(ed, W["edge_w1"][l][2 * H:]) + W["edge_b1"][l], 0))
        ed = _bf(mm(h, W["edge_w2"][l]) + W["edge_b2"][l])    # e_new for dummy
        sneg.append(-(ed.astype(np.float32) @ _bf(W["node_w1"][l][H:]).astype(np.float32)))

    wbf = np.zeros((P, pb.cols), BF16)
    wf32 = np.zeros((P, pf.cols), np.float32)

    def setb(name, arr):
        off, rows, ncols = pb.items[name]
        wbf[:rows, off:off + ncols] = _bf(arr).reshape(rows, ncols)

    def setf(name, arr):
        off, rows, ncols = pf.items[name]
        wf32[:rows, off:off + ncols] = np.asarray(arr, np.float32).reshape(rows, ncols)

    setb("ident", np.eye(P))
    setb("zeros", np.zeros((16, P)))
    setb("ee_w1", W["ee_w1"])
    setb("ee_w2", W["ee_w2"])
    setb("ne_w2", W["ne_w2"])
    for l in range(cfg.L):
        setb(f"ew1a{l}", W["edge_w1"][l][:H])
        setb(f"ew1b{l}", W["edge_w1"][l][H:2 * H])
        setb(f"ew1c{l}", W["edge_w1"][l][2 * H:])
        setb(f"ew2{l}", W["edge_w2"][l])
        setb(f"nw1a{l}", W["node_w1"][l][:H])
        setb(f"nw1b{l}", W["node_w1"][l][H:])
        setb(f"nw2{l}", W["node_w2"][l])
        setb(f"sneg{l}", sneg[l])
    setb("nd_w2", W["nd_w2"])
    setb("ndum", ndum)

    setf("ne_w1", W["ne_w1"])
    setf("nd_w1", W["nd_w1"])
    setf("ne_b1", W["ne_b1"]); setf("ne_b2", W["ne_b2"])
    setf("ee_b1", W["ee_b1"]); setf("ee_b2", W["ee_b2"])
    setf("ndb1", W["nd_b1"])
    ndb2 = np.zeros(P, np.float32); ndb2[:cfg.NO] = W["nd_b2"]
    setf("ndb2", ndb2)
    for l in range(cfg.L):
        setf(f"eb1{l}", W["edge_b1"][l]); setf(f"eb2{l}", W["edge_b2"][l])
        setf(f"nb1{l}", W["node_b1"][l]); setf(f"nb2{l}", W["node_b2"][l])
    return wbf, wf32


# ----------------------------------------------------------------------------
# Bass program
# ----------------------------------------------------------------------------

FLAGS = set()


def build_program(cfg, pb, pf):
    nc = bacc.Bacc("TRN2", target_bir_lowering=False,
                   debug=not axon_active(), num_devices=NCORES)
    L, NSH, NSH_pad, E_pad, NCH = cfg.L, cfg.NSH, cfg.NSH_pad, cfg.E_pad, cfg.NCH
    NT = NSH_pad // T           # node tiles
    NB = NSH_pad // P           # node transpose blocks
    GIDX = G // 16

    xin_d = nc.dram_tensor("xin", [cfg.NI, NSH_pad], F32, kind="ExternalInput")
    attrT_d = nc.dram_tensor("attrT", [cfg.EI, E_pad], BF, kind="ExternalInput")
    idxr_d = nc.dram_tensor("idx_row", [P, E_pad // 16], I16, kind="ExternalInput")
    idxc_d = nc.dram_tensor("idx_col", [P, E_pad // 16], I16, kind="ExternalInput")
    wbf_d = nc.dram_tensor("wbf", [P, pb.cols], BF, kind="ExternalInput")
    wf32_d = nc.dram_tensor("wf32", [P, pf.cols], F32, kind="ExternalInput")
    out_d = nc.dram_tensor("out", [cfg.NO, NSH], F32, kind="ExternalOutput")

    with tile.TileContext(nc) as tc:
        with (
            tc.tile_pool(name="persist", bufs=1) as pp,
            tc.tile_pool(name="work", bufs=2) as wp,
            tc.tile_pool(name="psum", bufs=1, space="PSUM") as psp,
            tc.tile_pool(name="dram", bufs=1, space="DRAM") as dp,
        ):
            # ---- persistent SBUF ----
            wbf = pp.tile([P, pb.cols], BF)
            wf32 = pp.tile([P, pf.cols], F32)
            idxr = pp.tile([P, E_pad // 16], I16)
            idxc = pp.tile([P, E_pad // 16], I16)
            e_car = pp.tile([P, E_pad], BF)
            x_fp = pp.tile([P, NSH_pad], F32)
            x_bf = pp.tile([P, NSH_pad], BF)
            agg_f = pp.tile([P, NSH_pad], F32)
            agg_bf = pp.tile([P, NSH_pad], BF)
            x_rows = pp.tile([P, NB, P], BF)

            nc.sync.dma_start(out=wbf[:], in_=wbf_d[:])
            nc.sync.dma_start(out=wf32[:], in_=wf32_d[:])
            nc.sync.dma_start(out=idxr[:], in_=idxr_d[:])
            nc.sync.dma_start(out=idxc[:], in_=idxc_d[:])

            def wb(name, rows=P, cols=H):
                o = pb.off(name)
                return wbf[0:rows, o:o + cols]

            def wf(name, rows=P, cols=H):
                o = pf.off(name)
                return wf32[0:rows, o:o + cols]

            def bias(name):
                o = pf.off(name)
                return wf32[:, o:o + 1]

            ident = wb("ident")

            # ---- DRAM: per-layer x tables (Shared = single AG writer each) ----
            x_tbls = [dp.tile([cfg.NTOT, H], BF, addr_space="Shared",
                              name=f"x_tbl{li}") for li in range(L)]
            ag_in = dp.tile([cfg.STRIDE, H], BF)
            # zero row inside every core's block tail (dummy-slot gather target)
            nc.sync.dma_start(out=ag_in[NSH_pad:cfg.STRIDE, :],
                              in_=wb("zeros", rows=16, cols=P))

            nc.vector.memset(agg_f[:, NSH:NSH_pad], 0.0)

            num_reg = nc.gpsimd.to_reg(G)

            relu = mybir.ActivationFunctionType.Relu
            copyf = mybir.ActivationFunctionType.Copy
            add = mybir.AluOpType.add

            def mlp_tile(dst, dst_sl, parts, b1ap, w2, b2ap, n):
                """dst[dst_sl] = (relu(sum_i w_i^T @ rhs_i + b1) )^T @ w2 + b2,
                feature-major, one [*, n] tile. parts: list of (lhsT_ap, rhs_ap)."""
                h_ps = psp.tile([P, T], F32, tag="ps512", bufs=5)
                for i, (lw, rr) in enumerate(parts):
                    nc.tensor.matmul(out=h_ps[:, :n], lhsT=lw, rhs=rr,
                                     start=(i == 0), stop=(i == len(parts) - 1))
                h_sb = wp.tile([P, T], BF, tag="h_sb", bufs=4)
                nc.scalar.activation(h_sb[:, :n], h_ps[:, :n], relu, bias=b1ap)
                o_ps = psp.tile([P, T], F32, tag="ps512", bufs=5)
                nc.tensor.matmul(out=o_ps[:w2.shape[1], :n], lhsT=w2, rhs=h_sb[:, :n],
                                 start=True, stop=True)
                # dst = o_ps + b2 (per-partition bias), with dtype cast
                nc.vector.tensor_scalar(out=dst[dst_sl], in0=o_ps[:w2.shape[1], :n],
                                        scalar1=b2ap, scalar2=None, op0=add)

            # ---- node encoder ----
            for t in range(NT):
                sl = np.s_[:, t * T:(t + 1) * T]
                xt = wp.tile([cfg.NI, T], F32, tag="attr", bufs=2)
                nc.sync.dma_start(out=xt[:], in_=xin_d[:, t * T:(t + 1) * T])
                mlp_tile(x_fp, sl, [(wf("ne_w1", rows=cfg.NI), xt[:])],
                         bias("ne_b1"), wb("ne_w2"), bias("ne_b2"), T)
                nc.vector.tensor_copy(out=x_bf[sl], in_=x_fp[sl])

            # ---- x table publish (transpose + AG) ----
            def publish_x(li):
                if "notrans" in FLAGS:
                    nc.vector.memset(x_rows[:], 0.0)
                else:
                    for j in range(NB):
                        tp = psp.tile([P, P], BF, tag="pst", bufs=2)
                        nc.tensor.transpose(out=tp[:], in_=x_bf[:, j * P:(j + 1) * P],
                                            identity=ident)
                        nc.scalar.activation(x_rows[:, j, :], tp[:], copyf)
                ag_view = ag_in[0:NSH_pad, :].rearrange("(j p) q -> p j q", p=P)
                nc.sync.dma_start(out=ag_view, in_=x_rows[:])
                nc.gpsimd.collective_compute(
                    "AllGather", mybir.AluOpType.bypass,
                    replica_groups=[list(range(NCORES))],
                    ins=[ag_in[:]], outs=[x_tbls[li][:]])

            publish_x(0)

            # ---- edge encoder ----
            for k in range(NCH):
                at = wp.tile([cfg.EI, G], BF, tag="attr", bufs=2)
                nc.sync.dma_start(out=at[:], in_=attrT_d[:, k * G:(k + 1) * G])
                for t in range(G // T):
                    s0 = k * G + t * T
                    mlp_tile(e_car, np.s_[:, s0:s0 + T],
                             [(wb("ee_w1", rows=cfg.EI), at[:, t * T:(t + 1) * T])],
                             bias("ee_b1"), wb("ee_w2"), bias("ee_b2"), T)

            # ---- message-passing layers ----
            for l in range(L):
                for k in range(NCH):
                    xr = wp.tile([P, 1, G], BF, tag="xr", bufs=2)
                    xc = wp.tile([P, 1, G], BF, tag="xc", bufs=2)
                    if "nogather" in FLAGS:
                        nc.vector.memset(xr[:], 0.0)
                        nc.vector.memset(xc[:], 0.0)
                    else:
                        nc.gpsimd.dma_gather(xr[:], x_tbls[l][:], idxr[:, k * GIDX:(k + 1) * GIDX],
                                             G, num_reg, H, transpose=True,
                                             single_packet=False)
                        nc.gpsimd.dma_gather(xc[:], x_tbls[l][:], idxc[:, k * GIDX:(k + 1) * GIDX],
                                             G, num_reg, H, transpose=True,
                                             single_packet=False)
                    for t in range(G // T):
                        s0 = k * G + t * T
                        mlp_tile(e_car, np.s_[:, s0:s0 + T],
                                 [(wb(f"ew1a{l}"), xr[:, 0, t * T:(t + 1) * T]),
                                  (wb(f"ew1b{l}"), xc[:, 0, t * T:(t + 1) * T]),
                                  (wb(f"ew1c{l}"), e_car[:, s0:s0 + T])],
                                 bias(f"eb1{l}"), wb(f"ew2{l}"), bias(f"eb2{l}"), T)

                # scatter-add: windowed segment reduce
                if "noreduce" in FLAGS:
                    nc.vector.memset(agg_f[:, 0:NSH], 0.0)
                else:
                    for (so, no, n, d) in cfg.groups:
                        src = e_car[:, so:so + n * d].rearrange("p (w d) -> p w d", d=d)
                        nc.vector.tensor_reduce(out=agg_f[:, no:no + n], in_=src,
                                                axis=mybir.AxisListType.X, op=add)
                nc.vector.tensor_copy(out=agg_bf[:], in_=agg_f[:])

                # node MLP + residual
                for t in range(NT):
                    sl = np.s_[:, t * T:(t + 1) * T]
                    h_ps = psp.tile([P, T], F32, tag="ps512", bufs=5)
                    nc.tensor.matmul(out=h_ps[:], lhsT=wb(f"nw1a{l}"), rhs=x_bf[sl],
                                     start=True, stop=False)
                    nc.tensor.matmul(out=h_ps[:], lhsT=wb(f"nw1b{l}"), rhs=agg_bf[sl],
                                     start=False, stop=("nok1" in FLAGS))
                    if "nok1" not in FLAGS:
                        nc.tensor.matmul(out=h_ps[:], lhsT=wb(f"sneg{l}", rows=1),
                                         rhs=wb("ndum", rows=1, cols=NSH_pad)[:, t * T:(t + 1) * T],
                                         start=False, stop=True)
                    h_sb = wp.tile([P, T], BF, tag="h_sb", bufs=4)
                    nc.scalar.activation(h_sb[:], h_ps[:], relu, bias=bias(f"nb1{l}"))
                    o_ps = psp.tile([P, T], F32, tag="ps512", bufs=5)
                    nc.tensor.matmul(out=o_ps[:], lhsT=wb(f"nw2{l}"), rhs=h_sb[:],
                                     start=True, stop=True)
                    # x_fp = (o_ps + nb2) + x_fp  (residual, fp32)
                    if "nostt" in FLAGS:
                        nc.vector.tensor_copy(out=x_fp[sl], in_=o_ps[:])
                    else:
                        nc.vector.scalar_tensor_tensor(out=x_fp[sl], in0=o_ps[:],
                                                       scalar=bias(f"nb2{l}"), in1=x_fp[sl],
                                                       op0=add, op1=add)
                    nc.vector.tensor_copy(out=x_bf[sl], in_=x_fp[sl])

                if l < L - 1:
                    publish_x(l + 1)

            # ---- decoder ----
            for t in range(NT):
                sl = np.s_[:, t * T:(t + 1) * T]
                h_ps = psp.tile([P, T], F32, tag="ps512", bufs=5)
                nc.tensor.matmul(out=h_ps[:], lhsT=wf("nd_w1"), rhs=x_fp[sl],
                                 start=True, stop=True)
                h_sb = wp.tile([P, T], BF, tag="h_sb", bufs=4)
                nc.scalar.activation(h_sb[:], h_ps[:], relu, bias=bias("ndb1"))
                o_ps = psp.tile([cfg.NO, T], F32, tag="ps512", bufs=5)
                nc.tensor.matmul(out=o_ps[:], lhsT=wb("nd_w2", cols=cfg.NO), rhs=h_sb[:],
                                 start=True, stop=True)
                ot = wp.tile([cfg.NO, T], F32, tag="attr", bufs=2)
                nc.vector.tensor_scalar(out=ot[:], in0=o_ps[:],
                                        scalar1=bias("ndb2")[0:cfg.NO, :],
                                        scalar2=None, op0=add)
                n_real = min(T, NSH - t * T)
                if n_real > 0:
                    nc.sync.dma_start(out=out_d[:, t * T:t * T + n_real],
                                      in_=ot[:, :n_real])

    nc.compile()
    return nc


# ----------------------------------------------------------------------------
# Driver
# ----------------------------------------------------------------------------

def make_in_maps(cfg, pb, pf, inputs, per_core):
    in_maps = []
    for c in range(NCORES):
        pc = per_core[c]
        wbf, wf32 = fill_packs(cfg, pb, pf, inputs, pc["ndum"])
        in_maps.append({
            "xin": pc["xin"],
            "attrT": _bf(pc["attrT"]),
            "idx_row": _wrap_idx(pc["row_idx"]),
            "idx_col": _wrap_idx(pc["col_idx"]),
            "wbf": wbf,
            "wf32": wf32,
        })
    return in_maps


def assemble_output(cfg, per_core, results):
    out = np.zeros((cfg.N, cfg.NO), np.float32)
    for c in range(NCORES):
        o = results[c]["out"]                      # [NO, NSH]
        out[per_core[c]["nodes_c"]] = o.T
    return out


_cache = {}


def kernel(**inputs) -> np.ndarray:
    cfg, per_core = preprocess(inputs)
    pb, pf = make_packs(cfg)
    key = (cfg.N, cfg.E, cfg.L, cfg.E_pad, tuple(cfg.groups))
    if key not in _cache:
        _cache[key] = build_program(cfg, pb, pf)
    nc = _cache[key]
    in_maps = make_in_maps(cfg, pb, pf, inputs, per_core)
    res = run_bass_kernel_spmd(nc, in_maps, list(range(NCORES)))
    return assemble_output(cfg, per_core, res.results)


if __name__ == "__main__":
    # quick self-drive with random mini inputs
    rng = np.random.default_rng(0)
    N, E, L, NI, EI, NO = 1024, 8192, 2, 6, 3, 3
    Hd = 128
    inputs = dict(
        x=rng.standard_normal((N, NI)).astype(np.float32),
        edge_attr=rng.standard_normal((E, EI)).astype(np.float32),
        edge_index=rng.integers(0, N, (2, E)).astype(np.int32),
        batch=np.zeros(N, np.int32),
        ne_w1=rng.standard_normal((NI, Hd)).astype(np.float32) / np.sqrt(NI),
        ne_b1=np.zeros(Hd, np.float32),
        ne_w2=rng.standard_normal((Hd, Hd)).astype(np.float32) / np.sqrt(Hd),
        ne_b2=np.zeros(Hd, np.float32),
        ee_w1=rng.standard_normal((EI, Hd)).astype(np.float32) / np.sqrt(EI),
        ee_b1=np.zeros(Hd, np.float32),
        ee_w2=rng.standard_normal((Hd, Hd)).astype(np.float32) / np.sqrt(Hd),
        ee_b2=np.zeros(Hd, np.float32),
        edge_w1=rng.standard_normal((L, 3 * Hd, Hd)).astype(np.float32) / np.sqrt(3 * Hd),
        edge_b1=np.zeros((L, Hd), np.float32),
        edge_w2=rng.standard_normal((L, Hd, Hd)).astype(np.float32) / np.sqrt(Hd),
        edge_b2=np.zeros((L, Hd), np.float32),
        node_w1=rng.standard_normal((L, 2 * Hd, Hd)).astype(np.float32) / np.sqrt(2 * Hd),
        node_b1=np.zeros((L, Hd), np.float32),
        node_w2=rng.standard_normal((L, Hd, Hd)).astype(np.float32) / np.sqrt(Hd),
        node_b2=np.zeros((L, Hd), np.float32),
        nd_w1=rng.standard_normal((Hd, Hd)).astype(np.float32) / np.sqrt(Hd),
        nd_b1=np.zeros(Hd, np.float32),
        nd_w2=rng.standard_normal((Hd, NO)).astype(np.float32) / np.sqrt(Hd),
        nd_b2=np.zeros(NO, np.float32),
    )
    got = kernel(**inputs)

    # numpy reference
    def mlp2(h, w1, b1, w2, b2):
        return np.maximum(h @ w1 + b1, 0.0) @ w2 + b2
    xx = mlp2(inputs["x"], inputs["ne_w1"], inputs["ne_b1"], inputs["ne_w2"], inputs["ne_b2"])
    e = mlp2(inputs["edge_attr"], inputs["ee_w1"], inputs["ee_b1"], inputs["ee_w2"], inputs["ee_b2"])
    r, cl = inputs["edge_index"][0], inputs["edge_index"][1]
    for l in range(L):
        msg = np.concatenate([xx[r], xx[cl], e], 1)
        e_new = mlp2(msg, inputs["edge_w1"][l], inputs["edge_b1"][l],
                     inputs["edge_w2"][l], inputs["edge_b2"][l])
        agg = np.zeros_like(xx)
        np.add.at(agg, r, e_new)
        x_new = mlp2(np.concatenate([xx, agg], 1), inputs["node_w1"][l],
                     inputs["node_b1"][l], inputs["node_w2"][l], inputs["node_b2"][l])
        xx, e = x_new + xx, e_new
    want = mlp2(xx, inputs["nd_w1"], inputs["nd_b1"], inputs["nd_w2"], inputs["nd_b2"])
    err = np.linalg.norm(got - want) / np.linalg.norm(want)
    print("mini rel l2 err:", err)
    print("max abs err:", np.abs(got - want).max(), "scale:", np.abs(want).max())


# revision 18
# speedup vs baseline: 1.6143x; 1.6143x over previous
"""MeshGraphNet on 8 Trainium2 NeuronCores (Bass/Tile, SPMD).

Strategy
--------
- Nodes are ranked by scatter-degree (deg over edge_index[0]) and dealt
  round-robin to 8 cores, so every core holds NSH nodes with a near-identical
  degree profile. Edges live on the core that owns their `row` endpoint
  (the scatter-add destination), sorted by destination, with each node's
  edges padded to a common per-position window size D[i] (the max degree of
  the 8 nodes dealt at position i). That makes the program layout (window
  sizes, reduce regions) identical across cores: SPMD with per-core data.
- All activations feature-major in SBUF ([H=128 partitions, items free]).
  Edge features for the whole core-shard stay resident in SBUF in bf16.
- Gathers of x[row]/x[col] use the dma_gather(transpose=True) custom DMA
  against a bf16 x-table in DRAM, yielding feature-major tiles directly.
- scatter-add == windowed segment-reduce over the destination-sorted edge
  features (DVE tensor_reduce per equal-window region). Dummy-slot
  contributions are cancelled with a host-computed rank-1 correction
  (K=1 matmul) folded into the node MLP.
- Per layer, updated node features are AllGather'd into the DRAM x-table.
- fp32 is kept for: node residual stream, encoder/decoder first matmuls,
  PSUM accumulation, biases. Everything else bf16.
"""

import numpy as np
import ml_dtypes

import concourse.bacc as bacc
import concourse.mybir as mybir
import concourse.tile as tile
from concourse.bass_utils import run_bass_kernel_spmd
from concourse._compat import axon_active

BF16 = ml_dtypes.bfloat16
NCORES = 8
H = 128
T = 512          # matmul tile (free dim)
G = 4096         # gather chunk (indices per dma_gather)
P = 128

F32 = mybir.dt.float32
BF = mybir.dt.bfloat16
I16 = mybir.dt.int16


def _bf(a):
    return np.asarray(a, np.float32).astype(BF16)


class Cfg:
    def __init__(self, N, E, L, NI, EI, NO):
        self.N, self.E, self.L = N, E, L
        self.NI, self.EI, self.NO = NI, EI, NO
        assert N % NCORES == 0
        self.NSH = N // NCORES                      # nodes per core
        self.NSH_pad = -(-self.NSH // T) * T        # padded to tile multiple
        self.STRIDE = self.NSH_pad + 16             # per-core block in x-table
        self.NTOT = NCORES * self.STRIDE            # x-table rows
        self.ZERO_ROW = self.NSH_pad                # core-0 zero row (dummies)
        # filled by preprocess:
        self.E_pad = None
        self.NCH = None
        self.groups = None                          # [(slot_off, node_off, n, d)]


# ----------------------------------------------------------------------------
# Host preprocessing
# ----------------------------------------------------------------------------

def preprocess(inputs):
    x = np.asarray(inputs["x"], np.float32)
    edge_attr = np.asarray(inputs["edge_attr"], np.float32)
    ei = np.asarray(inputs["edge_index"]).astype(np.int64)
    row, col = ei[0], ei[1]

    N, NI = x.shape
    E, EI = edge_attr.shape
    L = np.asarray(inputs["edge_w1"]).shape[0]
    NO = np.asarray(inputs["nd_w2"]).shape[1]
    cfg = Cfg(N, E, L, NI, EI, NO)
    NSH, NSH_pad = cfg.NSH, cfg.NSH_pad

    deg = np.bincount(row, minlength=N)
    order = np.argsort(-deg, kind="stable")         # node ids, degree desc
    rank = np.empty(N, np.int64)
    rank[order] = np.arange(N)
    core_of = rank % NCORES
    pos_of = rank // NCORES
    permpos = core_of * cfg.STRIDE + pos_of         # x-table row per node

    D = np.maximum(deg[order[0::NCORES]], 1)        # common window sizes [NSH]
    Woff = np.concatenate([[0], np.cumsum(D)])
    S = int(Woff[-1])
    cfg.E_pad = -(-S // G) * G
    cfg.NCH = cfg.E_pad // G

    # reduce groups: runs of equal D
    groups = []
    i = 0
    while i < NSH:
        j = i
        while j < NSH and D[j] == D[i]:
            j += 1
        groups.append((int(Woff[i]), i, j - i, int(D[i])))
        i = j
    cfg.groups = groups

    # per-gather-chunk broadcast-expansion plan for x[row]:
    # pieces (rel_slot, node_off, n_windows, win_len) per chunk
    plans = [[] for _ in range(cfg.E_pad // G)]
    for (so, no, n, d) in groups:
        lo, hi = so, so + n * d
        k0, k1 = lo // G, (hi - 1) // G
        for k in range(k0, k1 + 1):
            a, b = max(lo, k * G), min(hi, (k + 1) * G)
            w = (a - so) // d
            off = (a - so) % d
            if off:
                p0 = min(b - a, d - off)
                plans[k].append((a - k * G, no + w, 1, p0))
                a += p0
                w += 1
            nf = (b - a) // d
            if nf:
                plans[k].append((a - k * G, no + w, nf, d))
                a += nf * d
                w += nf
            if a < b:
                plans[k].append((a - k * G, no + w, 1, b - a))
    # zero-fill pieces for trailing pad slots (node_off = -1 -> memset)
    for k in range(cfg.E_pad // G):
        a = max(S, k * G)
        if a < (k + 1) * G:
            plans[k].append((a - k * G, -1, 1, (k + 1) * G - a))
    cfg.plans = plans

    # --- per-core edge layout ---
    edge_order = np.argsort(row, kind="stable")
    start = np.concatenate([[0], np.cumsum(deg)])   # edges of node n: edge_order[start[n]:start[n]+deg[n]]

    per_core = []
    for c in range(NCORES):
        nodes_c = order[NCORES * np.arange(NSH) + c]
        degs_c = deg[nodes_c]
        tot = int(degs_c.sum())
        cum0 = np.concatenate([[0], np.cumsum(degs_c)])[:-1]
        within = np.arange(tot) - np.repeat(cum0, degs_c)
        e_ids = edge_order[np.repeat(start[nodes_c], degs_c) + within]
        slots = np.repeat(Woff[:NSH], degs_c) + within

        col_idx = np.full(cfg.E_pad, cfg.ZERO_ROW, np.int32)
        row_idx = np.full(cfg.E_pad, cfg.ZERO_ROW, np.int32)
        attrT = np.zeros((EI, cfg.E_pad), np.float32)
        col_idx[slots] = permpos[col[e_ids]]
        row_idx[slots] = np.repeat(permpos[nodes_c], degs_c)
        attrT[:, slots] = edge_attr[e_ids].T

        ndum = np.zeros(cfg.NSH_pad, np.float32)
        ndum[:NSH] = D - degs_c

        xin = np.zeros((NI, NSH_pad), np.float32)
        xin[:, :NSH] = x[nodes_c].T

        per_core.append(dict(col_idx=col_idx, row_idx=row_idx, attrT=attrT,
                             ndum=ndum, xin=xin, nodes_c=nodes_c))

    return cfg, per_core


def _wrap_idx(idx):
    """[E_pad] int -> [128, E_pad//16] int16 (wrapped + replicated per Q7 core)."""
    n = idx.shape[0]
    assert n % 16 == 0
    w = np.asarray(idx, np.int16).reshape(n // 16, 16).T    # [16, n/16]
    return np.tile(w, (8, 1))


# ----------------------------------------------------------------------------
# Weight packing (shared between builder offsets and host fill)
# ----------------------------------------------------------------------------

class Pack:
    def __init__(self):
        self.cols = 0
        self.items = {}   # name -> (off, rows, ncols)

    def add(self, name, rows, ncols):
        self.items[name] = (self.cols, rows, ncols)
        self.cols += ncols

    def off(self, name):
        return self.items[name][0]


def make_packs(cfg):
    pb = Pack()           # bf16 pack [128, pb.cols]
    pb.add("ident", P, P)
    pb.add("zeros", 16, P)
    pb.add("ee_w1", cfg.EI, H)
    pb.add("ee_w2", H, H)
    pb.add("ne_w2", H, H)
    for l in range(cfg.L):
        pb.add(f"ew1a{l}", H, H)
        pb.add(f"ew1b{l}", H, H)
        pb.add(f"ew1c{l}", H, H)
        pb.add(f"ew2{l}", H, H)
        pb.add(f"nw1a{l}", H, H)
        pb.add(f"nw1b{l}", H, H)
        pb.add(f"nw2{l}", H, H)
    pb.add("nd_w2", H, cfg.NO)

    pf = Pack()           # f32 pack [128, pf.cols]
    pf.add("ne_w1", cfg.NI, H)
    pf.add("nd_w1", H, H)
    for nm in ["ne_b1", "ne_b2", "ee_b1", "ee_b2", "ndb1", "ndb2"]:
        pf.add(nm, H, 1)
    for l in range(cfg.L):
        for nm in ["eb1", "eb2", "nb1", "nb2"]:
            pf.add(f"{nm}{l}", H, 1)
    return pb, pf


def fill_packs(cfg, pb, pf, inputs):
    """Returns (wbf [128, pb.cols] bf16, wf32 [128, pf.cols] f32)."""
    W = {k: np.asarray(v, np.float32) for k, v in inputs.items()
         if k not in ("x", "edge_attr", "edge_index", "batch")}

    wbf = np.zeros((P, pb.cols), BF16)
    wf32 = np.zeros((P, pf.cols), np.float32)

    def setb(name, arr):
        off, rows, ncols = pb.items[name]
        wbf[:rows, off:off + ncols] = _bf(arr).reshape(rows, ncols)

    def setf(name, arr):
        off, rows, ncols = pf.items[name]
        wf32[:rows, off:off + ncols] = np.asarray(arr, np.float32).reshape(rows, ncols)

    setb("ident", np.eye(P))
    setb("zeros", np.zeros((16, P)))
    setb("ee_w1", W["ee_w1"])
    setb("ee_w2", W["ee_w2"])
    setb("ne_w2", W["ne_w2"])
    for l in range(cfg.L):
        setb(f"ew1a{l}", W["edge_w1"][l][:H])
        setb(f"ew1b{l}", W["edge_w1"][l][H:2 * H])
        setb(f"ew1c{l}", W["edge_w1"][l][2 * H:])
        setb(f"ew2{l}", W["edge_w2"][l])
        setb(f"nw1a{l}", W["node_w1"][l][:H])
        setb(f"nw1b{l}", W["node_w1"][l][H:])
        setb(f"nw2{l}", W["node_w2"][l])
    setb("nd_w2", W["nd_w2"])

    setf("ne_w1", W["ne_w1"])
    setf("nd_w1", W["nd_w1"])
    setf("ne_b1", W["ne_b1"]); setf("ne_b2", W["ne_b2"])
    setf("ee_b1", W["ee_b1"]); setf("ee_b2", W["ee_b2"])
    setf("ndb1", W["nd_b1"])
    ndb2 = np.zeros(P, np.float32); ndb2[:cfg.NO] = W["nd_b2"]
    setf("ndb2", ndb2)
    for l in range(cfg.L):
        setf(f"eb1{l}", W["edge_b1"][l]); setf(f"eb2{l}", W["edge_b2"][l])
        setf(f"nb1{l}", W["node_b1"][l]); setf(f"nb2{l}", W["node_b2"][l])
    return wbf, wf32


# ----------------------------------------------------------------------------
# Bass program
# ----------------------------------------------------------------------------

FLAGS = set()


def build_program(cfg, pb, pf):
    nc = bacc.Bacc("TRN2", target_bir_lowering=False,
                   debug=not axon_active(), num_devices=NCORES,
                   num_swdge_queues=4)
    L, NSH, NSH_pad, E_pad, NCH = cfg.L, cfg.NSH, cfg.NSH_pad, cfg.E_pad, cfg.NCH
    NT = NSH_pad // T           # node tiles
    NB = NSH_pad // P           # node transpose blocks
    GIDX = G // 16

    xin_d = nc.dram_tensor("xin", [cfg.NI, NSH_pad], F32, kind="ExternalInput")
    attrT_d = nc.dram_tensor("attrT", [cfg.EI, E_pad], BF, kind="ExternalInput")
    idxc_d = nc.dram_tensor("idx_col", [P, E_pad // 16], I16, kind="ExternalInput")
    ndum_d = nc.dram_tensor("ndum", [P, NSH_pad], BF, kind="ExternalInput")
    wbf_d = nc.dram_tensor("wbf", [P, pb.cols], BF, kind="ExternalInput")
    wf32_d = nc.dram_tensor("wf32", [P, pf.cols], F32, kind="ExternalInput")
    out_d = nc.dram_tensor("out", [cfg.NO, NSH], F32, kind="ExternalOutput")

    with tile.TileContext(nc) as tc:
        with (
            tc.tile_pool(name="persist", bufs=1) as pp,
            tc.tile_pool(name="work", bufs=2) as wp,
            tc.tile_pool(name="psum", bufs=1, space="PSUM") as psp,
            tc.tile_pool(name="dram", bufs=1, space="DRAM") as dp,
        ):
            # ---- persistent SBUF ----
            wbf = pp.tile([P, pb.cols], BF)
            wf32 = pp.tile([P, pf.cols], F32)
            idxc = pp.tile([P, E_pad // 16], I16)
            ndum = pp.tile([P, NSH_pad], BF)
            vdum = pp.tile([P, NSH_pad], BF)
            e_car = pp.tile([P, E_pad], BF)
            x_fp = pp.tile([P, NSH_pad], F32)
            x_bf = pp.tile([P, NSH_pad], BF)
            agg_f = pp.tile([P, NSH_pad], F32)
            agg_bf = pp.tile([P, NSH_pad], BF)
            x_rows = pp.tile([P, NB, P], BF)

            nc.sync.dma_start(out=wbf[:], in_=wbf_d[:])
            nc.sync.dma_start(out=wf32[:], in_=wf32_d[:])
            nc.sync.dma_start(out=idxc[:], in_=idxc_d[:])
            nc.sync.dma_start(out=ndum[:], in_=ndum_d[:])

            def wb(name, rows=P, cols=H):
                o = pb.off(name)
                return wbf[0:rows, o:o + cols]

            def wf(name, rows=P, cols=H):
                o = pf.off(name)
                return wf32[0:rows, o:o + cols]

            def bias(name):
                o = pf.off(name)
                return wf32[:, o:o + 1]

            ident = wb("ident")

            # ---- DRAM: per-layer x tables (Shared = single AG writer each) ----
            x_tbls = [dp.tile([cfg.NTOT, H], BF, addr_space="Shared",
                              name=f"x_tbl{li}") for li in range(L)]
            ag_in = dp.tile([cfg.STRIDE, H], BF)
            # zero row inside every core's block tail (dummy-slot gather target)
            nc.sync.dma_start(out=ag_in[NSH_pad:cfg.STRIDE, :],
                              in_=wb("zeros", rows=16, cols=P))

            nc.vector.memset(agg_f[:, NSH:NSH_pad], 0.0)

            num_reg = nc.gpsimd.to_reg(G)

            relu = mybir.ActivationFunctionType.Relu
            copyf = mybir.ActivationFunctionType.Copy
            add = mybir.AluOpType.add

            def mlp_tile(dst, dst_sl, parts, b1ap, w2, b2ap, n):
                """dst[dst_sl] = (relu(sum_i w_i^T @ rhs_i + b1))^T @ w2 + b2,
                feature-major, one [*, n] tile. parts: list of (lhsT_ap, rhs_ap)."""
                h_ps = psp.tile([P, T], F32, tag="ps512", bufs=5)
                for i, (lw, rr) in enumerate(parts):
                    nc.tensor.matmul(out=h_ps[:, :n], lhsT=lw, rhs=rr,
                                     start=(i == 0), stop=(i == len(parts) - 1))
                h_sb = wp.tile([P, T], BF, tag="h_sb", bufs=4)
                nc.scalar.activation(h_sb[:, :n], h_ps[:, :n], relu, bias=b1ap)
                o_ps = psp.tile([P, T], F32, tag="ps512", bufs=5)
                nc.tensor.matmul(out=o_ps[:w2.shape[1], :n], lhsT=w2, rhs=h_sb[:, :n],
                                 start=True, stop=True)
                nc.vector.tensor_scalar(out=dst[dst_sl], in0=o_ps[:w2.shape[1], :n],
                                        scalar1=b2ap, scalar2=None, op0=add)

            # ---- node encoder ----
            for t in range(NT):
                sl = np.s_[:, t * T:(t + 1) * T]
                xt = wp.tile([cfg.NI, T], F32, tag="attr", bufs=2)
                nc.sync.dma_start(out=xt[:], in_=xin_d[:, t * T:(t + 1) * T])
                mlp_tile(x_fp, sl, [(wf("ne_w1", rows=cfg.NI), xt[:])],
                         bias("ne_b1"), wb("ne_w2"), bias("ne_b2"), T)
                nc.vector.tensor_copy(out=x_bf[sl], in_=x_fp[sl])

            # ---- x table publish (transpose + AG) ----
            def publish_x(li):
                if "notrans" in FLAGS:
                    nc.vector.memset(x_rows[:], 0.0)
                else:
                    for j in range(NB):
                        tp = psp.tile([P, P], BF, tag="pst", bufs=2)
                        nc.tensor.transpose(out=tp[:], in_=x_bf[:, j * P:(j + 1) * P],
                                            identity=ident)
                        nc.scalar.activation(x_rows[:, j, :], tp[:], copyf)
                ag_view = ag_in[0:NSH_pad, :].rearrange("(j p) q -> p j q", p=P)
                nc.sync.dma_start(out=ag_view, in_=x_rows[:])
                nc.gpsimd.collective_compute(
                    "AllGather", mybir.AluOpType.bypass,
                    replica_groups=[list(range(NCORES))],
                    ins=[ag_in[:]], outs=[x_tbls[li][:]])

            publish_x(0)

            # ---- edge encoder ----
            for k in range(NCH):
                at = wp.tile([cfg.EI, G], BF, tag="attr", bufs=2)
                nc.sync.dma_start(out=at[:], in_=attrT_d[:, k * G:(k + 1) * G])
                for t in range(G // T):
                    s0 = k * G + t * T
                    mlp_tile(e_car, np.s_[:, s0:s0 + T],
                             [(wb("ee_w1", rows=cfg.EI), at[:, t * T:(t + 1) * T])],
                             bias("ee_b1"), wb("ee_w2"), bias("ee_b2"), T)

            # v0: encoder output for an all-zero dummy edge (node-space mirror)
            zattr = pp.tile([cfg.EI, T], BF)
            nc.vector.memset(zattr[:], 0.0)
            for t in range(NT):
                mlp_tile(vdum, np.s_[:, t * T:(t + 1) * T],
                         [(wb("ee_w1", rows=cfg.EI), zattr[:])],
                         bias("ee_b1"), wb("ee_w2"), bias("ee_b2"), T)

            # ---- message-passing layers ----
            for l in range(L):
                for k in range(NCH):
                    xr = wp.tile([P, 1, G], BF, tag="xr", bufs=2)
                    xc = wp.tile([P, 1, G], BF, tag="xc", bufs=2)
                    if "nogather" in FLAGS:
                        nc.vector.memset(xr[:], 0.0)
                        nc.vector.memset(xc[:], 0.0)
                    else:
                        for (rel, no, n, dw) in cfg.plans[k]:
                            if no < 0:
                                nc.vector.memset(xr[:, 0, rel:rel + dw], 0.0)
                                continue
                            src = x_bf[:, no:no + n].rearrange(
                                "p (n o) -> p n o", o=1).to_broadcast([P, n, dw])
                            dst = xr[:, 0, rel:rel + n * dw].rearrange(
                                "p (n d) -> p n d", d=dw)
                            nc.vector.tensor_copy(out=dst, in_=src)
                        nc.gpsimd.dma_gather(xc[:], x_tbls[l][:], idxc[:, k * GIDX:(k + 1) * GIDX],
                                             G, num_reg, H, transpose=True,
                                             single_packet=False, queue_num=0)
                    for t in range(G // T):
                        s0 = k * G + t * T
                        mlp_tile(e_car, np.s_[:, s0:s0 + T],
                                 [(wb(f"ew1a{l}"), xr[:, 0, t * T:(t + 1) * T]),
                                  (wb(f"ew1b{l}"), xc[:, 0, t * T:(t + 1) * T]),
                                  (wb(f"ew1c{l}"), e_car[:, s0:s0 + T])],
                                 bias(f"eb1{l}"), wb(f"ew2{l}"), bias(f"eb2{l}"), T)

                # dummy-value chain in node space: v = edge_mlp([x, 0, v])
                for t in range(NT):
                    sl = np.s_[:, t * T:(t + 1) * T]
                    mlp_tile(vdum, sl,
                             [(wb(f"ew1a{l}"), x_bf[sl]),
                              (wb(f"ew1c{l}"), vdum[sl])],
                             bias(f"eb1{l}"), wb(f"ew2{l}"), bias(f"eb2{l}"), T)

                # scatter-add: windowed segment reduce
                if "noreduce" in FLAGS:
                    nc.vector.memset(agg_f[:, 0:NSH], 0.0)
                else:
                    for (so, no, n, d) in cfg.groups:
                        src = e_car[:, so:so + n * d].rearrange("p (w d) -> p w d", d=d)
                        nc.vector.tensor_reduce(out=agg_f[:, no:no + n], in_=src,
                                                axis=mybir.AxisListType.X, op=add)
                # remove dummy-slot contributions: agg -= ndum * v
                for t in range(NT):
                    sl = np.s_[:, t * T:(t + 1) * T]
                    vn = wp.tile([P, T], F32, tag="attr", bufs=2)
                    nc.vector.tensor_tensor(out=vn[:], in0=vdum[sl], in1=ndum[sl],
                                            op=mybir.AluOpType.mult)
                    nc.vector.tensor_tensor(out=agg_f[sl], in0=agg_f[sl], in1=vn[:],
                                            op=mybir.AluOpType.subtract)
                nc.vector.tensor_copy(out=agg_bf[:], in_=agg_f[:])

                # node MLP + residual
                for t in range(NT):
                    sl = np.s_[:, t * T:(t + 1) * T]
                    h_ps = psp.tile([P, T], F32, tag="ps512", bufs=5)
                    nc.tensor.matmul(out=h_ps[:], lhsT=wb(f"nw1a{l}"), rhs=x_bf[sl],
                                     start=True, stop=False)
                    nc.tensor.matmul(out=h_ps[:], lhsT=wb(f"nw1b{l}"), rhs=agg_bf[sl],
                                     start=False, stop=True)
                    h_sb = wp.tile([P, T], BF, tag="h_sb", bufs=4)
                    nc.scalar.activation(h_sb[:], h_ps[:], relu, bias=bias(f"nb1{l}"))
                    o_ps = psp.tile([P, T], F32, tag="ps512", bufs=5)
                    nc.tensor.matmul(out=o_ps[:], lhsT=wb(f"nw2{l}"), rhs=h_sb[:],
                                     start=True, stop=True)
                    # x_fp = (o_ps + nb2) + x_fp  (residual, fp32)
                    if "nostt" in FLAGS:
                        nc.vector.tensor_copy(out=x_fp[sl], in_=o_ps[:])
                    else:
                        nc.vector.scalar_tensor_tensor(out=x_fp[sl], in0=o_ps[:],
                                                       scalar=bias(f"nb2{l}"), in1=x_fp[sl],
                                                       op0=add, op1=add)
                    nc.vector.tensor_copy(out=x_bf[sl], in_=x_fp[sl])

                if l < L - 1:
                    publish_x(l + 1)

            # ---- decoder ----
            for t in range(NT):
                sl = np.s_[:, t * T:(t + 1) * T]
                h_ps = psp.tile([P, T], F32, tag="ps512", bufs=5)
                nc.tensor.matmul(out=h_ps[:], lhsT=wf("nd_w1"), rhs=x_fp[sl],
                                 start=True, stop=True)
                h_sb = wp.tile([P, T], BF, tag="h_sb", bufs=4)
                nc.scalar.activation(h_sb[:], h_ps[:], relu, bias=bias("ndb1"))
                o_ps = psp.tile([cfg.NO, T], F32, tag="ps512", bufs=5)
                nc.tensor.matmul(out=o_ps[:], lhsT=wb("nd_w2", cols=cfg.NO), rhs=h_sb[:],
                                 start=True, stop=True)
                ot = wp.tile([cfg.NO, T], F32, tag="attr", bufs=2)
                nc.vector.tensor_scalar(out=ot[:], in0=o_ps[:],
                                        scalar1=bias("ndb2")[0:cfg.NO, :],
                                        scalar2=None, op0=add)
                n_real = min(T, NSH - t * T)
                if n_real > 0:
                    nc.sync.dma_start(out=out_d[:, t * T:t * T + n_real],
                                      in_=ot[:, :n_real])

    nc.compile()
    return nc


# ----------------------------------------------------------------------------
# Driver
# ----------------------------------------------------------------------------

def make_in_maps(cfg, pb, pf, inputs, per_core):
    wbf, wf32 = fill_packs(cfg, pb, pf, inputs)
    in_maps = []
    for c in range(NCORES):
        pc = per_core[c]
        in_maps.append({
            "xin": pc["xin"],
            "attrT": _bf(pc["attrT"]),
            "ndum": np.tile(_bf(pc["ndum"])[None, :], (P, 1)),
            "idx_col": _wrap_idx(pc["col_idx"]),
            "wbf": wbf,
            "wf32": wf32,
        })
    return in_maps


def assemble_output(cfg, per_core, results):
    out = np.zeros((cfg.N, cfg.NO), np.float32)
    for c in range(NCORES):
        o = results[c]["out"]                      # [NO, NSH]
        out[per_core[c]["nodes_c"]] = o.T
    return out


_cache = {}


def kernel(**inputs) -> np.ndarray:
    cfg, per_core = preprocess(inputs)
    pb, pf = make_packs(cfg)
    key = (cfg.N, cfg.E, cfg.L, cfg.E_pad, tuple(cfg.groups))
    if key not in _cache:
        _cache[key] = build_program(cfg, pb, pf)
    nc = _cache[key]
    in_maps = make_in_maps(cfg, pb, pf, inputs, per_core)
    res = run_bass_kernel_spmd(nc, in_maps, list(range(NCORES)))
    return assemble_output(cfg, per_core, res.results)


if __name__ == "__main__":
    # quick self-drive with random mini inputs
    rng = np.random.default_rng(0)
    N, E, L, NI, EI, NO = 1024, 8192, 2, 6, 3, 3
    Hd = 128
    inputs = dict(
        x=rng.standard_normal((N, NI)).astype(np.float32),
        edge_attr=rng.standard_normal((E, EI)).astype(np.float32),
        edge_index=rng.integers(0, N, (2, E)).astype(np.int32),
        batch=np.zeros(N, np.int32),
        ne_w1=rng.standard_normal((NI, Hd)).astype(np.float32) / np.sqrt(NI),
        ne_b1=np.zeros(Hd, np.float32),
        ne_w2=rng.standard_normal((Hd, Hd)).astype(np.float32) / np.sqrt(Hd),
        ne_b2=np.zeros(Hd, np.float32),
        ee_w1=rng.standard_normal((EI, Hd)).astype(np.float32) / np.sqrt(EI),
        ee_b1=np.zeros(Hd, np.float32),
        ee_w2=rng.standard_normal((Hd, Hd)).astype(np.float32) / np.sqrt(Hd),
        ee_b2=np.zeros(Hd, np.float32),
        edge_w1=rng.standard_normal((L, 3 * Hd, Hd)).astype(np.float32) / np.sqrt(3 * Hd),
        edge_b1=np.zeros((L, Hd), np.float32),
        edge_w2=rng.standard_normal((L, Hd, Hd)).astype(np.float32) / np.sqrt(Hd),
        edge_b2=np.zeros((L, Hd), np.float32),
        node_w1=rng.standard_normal((L, 2 * Hd, Hd)).astype(np.float32) / np.sqrt(2 * Hd),
        node_b1=np.zeros((L, Hd), np.float32),
        node_w2=rng.standard_normal((L, Hd, Hd)).astype(np.float32) / np.sqrt(Hd),
        node_b2=np.zeros((L, Hd), np.float32),
        nd_w1=rng.standard_normal((Hd, Hd)).astype(np.float32) / np.sqrt(Hd),
        nd_b1=np.zeros(Hd, np.float32),
        nd_w2=rng.standard_normal((Hd, NO)).astype(np.float32) / np.sqrt(Hd),
        nd_b2=np.zeros(NO, np.float32),
    )
    got = kernel(**inputs)

    # numpy reference
    def mlp2(h, w1, b1, w2, b2):
        return np.maximum(h @ w1 + b1, 0.0) @ w2 + b2
    xx = mlp2(inputs["x"], inputs["ne_w1"], inputs["ne_b1"], inputs["ne_w2"], inputs["ne_b2"])
    e = mlp2(inputs["edge_attr"], inputs["ee_w1"], inputs["ee_b1"], inputs["ee_w2"], inputs["ee_b2"])
    r, cl = inputs["edge_index"][0], inputs["edge_index"][1]
    for l in range(L):
        msg = np.concatenate([xx[r], xx[cl], e], 1)
        e_new = mlp2(msg, inputs["edge_w1"][l], inputs["edge_b1"][l],
                     inputs["edge_w2"][l], inputs["edge_b2"][l])
        agg = np.zeros_like(xx)
        np.add.at(agg, r, e_new)
        x_new = mlp2(np.concatenate([xx, agg], 1), inputs["node_w1"][l],
                     inputs["node_b1"][l], inputs["node_w2"][l], inputs["node_b2"][l])
        xx, e = x_new + xx, e_new
    want = mlp2(xx, inputs["nd_w1"], inputs["nd_b1"], inputs["nd_w2"], inputs["nd_b2"])
    err = np.linalg.norm(got - want) / np.linalg.norm(want)
    print("mini rel l2 err:", err)
    print("max abs err:", np.abs(got - want).max(), "scale:", np.abs(want).max())


# revision 20
# speedup vs baseline: 1.6942x; 1.0495x over previous
"""MeshGraphNet on 8 Trainium2 NeuronCores (Bass/Tile, SPMD).

Strategy
--------
- Nodes are ranked by scatter-degree (deg over edge_index[0]) and dealt
  round-robin to 8 cores, so every core holds NSH nodes with a near-identical
  degree profile. Edges live on the core that owns their `row` endpoint
  (the scatter-add destination), sorted by destination, with each node's
  edges padded to a common per-position window size D[i] (the max degree of
  the 8 nodes dealt at position i). That makes the program layout (window
  sizes, reduce regions) identical across cores: SPMD with per-core data.
- All activations feature-major in SBUF ([H=128 partitions, items free]).
  Edge features for the whole core-shard stay resident in SBUF in bf16.
- Gathers of x[row]/x[col] use the dma_gather(transpose=True) custom DMA
  against a bf16 x-table in DRAM, yielding feature-major tiles directly.
- scatter-add == windowed segment-reduce over the destination-sorted edge
  features (DVE tensor_reduce per equal-window region). Dummy-slot
  contributions are cancelled with a host-computed rank-1 correction
  (K=1 matmul) folded into the node MLP.
- Per layer, updated node features are AllGather'd into the DRAM x-table.
- fp32 is kept for: node residual stream, encoder/decoder first matmuls,
  PSUM accumulation, biases. Everything else bf16.
"""

import numpy as np
import ml_dtypes

import concourse.bacc as bacc
import concourse.mybir as mybir
import concourse.tile as tile
from concourse.bass_utils import run_bass_kernel_spmd
from concourse._compat import axon_active

BF16 = ml_dtypes.bfloat16
NCORES = 8
H = 128
T = 512          # matmul tile (free dim)
G = 4096         # gather chunk (indices per dma_gather)
P = 128

F32 = mybir.dt.float32
BF = mybir.dt.bfloat16
I16 = mybir.dt.int16


def _bf(a):
    return np.asarray(a, np.float32).astype(BF16)


class Cfg:
    def __init__(self, N, E, L, NI, EI, NO):
        self.N, self.E, self.L = N, E, L
        self.NI, self.EI, self.NO = NI, EI, NO
        assert N % NCORES == 0
        self.NSH = N // NCORES                      # nodes per core
        self.NSH_pad = -(-self.NSH // T) * T        # padded to tile multiple
        self.STRIDE = self.NSH_pad + 16             # per-core block in x-table
        self.NTOT = NCORES * self.STRIDE            # x-table rows
        self.ZERO_ROW = self.NSH_pad                # core-0 zero row (dummies)
        # filled by preprocess:
        self.E_pad = None
        self.NCH = None
        self.groups = None                          # [(slot_off, node_off, n, d)]


# ----------------------------------------------------------------------------
# Host preprocessing
# ----------------------------------------------------------------------------

def preprocess(inputs):
    x = np.asarray(inputs["x"], np.float32)
    edge_attr = np.asarray(inputs["edge_attr"], np.float32)
    ei = np.asarray(inputs["edge_index"]).astype(np.int64)
    row, col = ei[0], ei[1]

    N, NI = x.shape
    E, EI = edge_attr.shape
    L = np.asarray(inputs["edge_w1"]).shape[0]
    NO = np.asarray(inputs["nd_w2"]).shape[1]
    cfg = Cfg(N, E, L, NI, EI, NO)
    cfg.zero_bias = all(
        not np.any(np.asarray(inputs[k]))
        for k in ("ne_b1", "ne_b2", "ee_b1", "ee_b2", "edge_b1", "edge_b2",
                  "node_b1", "node_b2", "nd_b1", "nd_b2"))
    NSH, NSH_pad = cfg.NSH, cfg.NSH_pad

    deg = np.bincount(row, minlength=N)
    order = np.argsort(-deg, kind="stable")         # node ids, degree desc
    rank = np.empty(N, np.int64)
    rank[order] = np.arange(N)
    core_of = rank % NCORES
    pos_of = rank // NCORES
    permpos = core_of * cfg.STRIDE + pos_of         # x-table row per node

    D = np.maximum(deg[order[0::NCORES]], 1)        # common window sizes [NSH]
    Woff = np.concatenate([[0], np.cumsum(D)])
    S = int(Woff[-1])
    cfg.E_pad = -(-S // G) * G
    cfg.NCH = cfg.E_pad // G

    # reduce groups: runs of equal D
    groups = []
    i = 0
    while i < NSH:
        j = i
        while j < NSH and D[j] == D[i]:
            j += 1
        groups.append((int(Woff[i]), i, j - i, int(D[i])))
        i = j
    cfg.groups = groups

    # per-gather-chunk broadcast-expansion plan for x[row]:
    # pieces (rel_slot, node_off, n_windows, win_len) per chunk
    plans = [[] for _ in range(cfg.E_pad // G)]
    for (so, no, n, d) in groups:
        lo, hi = so, so + n * d
        k0, k1 = lo // G, (hi - 1) // G
        for k in range(k0, k1 + 1):
            a, b = max(lo, k * G), min(hi, (k + 1) * G)
            w = (a - so) // d
            off = (a - so) % d
            if off:
                p0 = min(b - a, d - off)
                plans[k].append((a - k * G, no + w, 1, p0))
                a += p0
                w += 1
            nf = (b - a) // d
            if nf:
                plans[k].append((a - k * G, no + w, nf, d))
                a += nf * d
                w += nf
            if a < b:
                plans[k].append((a - k * G, no + w, 1, b - a))
    # zero-fill pieces for trailing pad slots (node_off = -1 -> memset)
    for k in range(cfg.E_pad // G):
        a = max(S, k * G)
        if a < (k + 1) * G:
            plans[k].append((a - k * G, -1, 1, (k + 1) * G - a))
    cfg.plans = plans

    # --- per-core edge layout ---
    edge_order = np.argsort(row, kind="stable")
    start = np.concatenate([[0], np.cumsum(deg)])   # edges of node n: edge_order[start[n]:start[n]+deg[n]]

    per_core = []
    for c in range(NCORES):
        nodes_c = order[NCORES * np.arange(NSH) + c]
        degs_c = deg[nodes_c]
        tot = int(degs_c.sum())
        cum0 = np.concatenate([[0], np.cumsum(degs_c)])[:-1]
        within = np.arange(tot) - np.repeat(cum0, degs_c)
        e_ids = edge_order[np.repeat(start[nodes_c], degs_c) + within]
        slots = np.repeat(Woff[:NSH], degs_c) + within

        col_idx = np.full(cfg.E_pad, cfg.ZERO_ROW, np.int32)
        row_idx = np.full(cfg.E_pad, cfg.ZERO_ROW, np.int32)
        attrT = np.zeros((EI, cfg.E_pad), np.float32)
        col_idx[slots] = permpos[col[e_ids]]
        row_idx[slots] = np.repeat(permpos[nodes_c], degs_c)
        attrT[:, slots] = edge_attr[e_ids].T

        ndum = np.zeros(cfg.NSH_pad, np.float32)
        ndum[:NSH] = D - degs_c

        xin = np.zeros((NI, NSH_pad), np.float32)
        xin[:, :NSH] = x[nodes_c].T

        per_core.append(dict(col_idx=col_idx, row_idx=row_idx, attrT=attrT,
                             ndum=ndum, xin=xin, nodes_c=nodes_c))

    return cfg, per_core


def _wrap_idx(idx):
    """[E_pad] int -> [128, E_pad//16] int16 (wrapped + replicated per Q7 core)."""
    n = idx.shape[0]
    assert n % 16 == 0
    w = np.asarray(idx, np.int16).reshape(n // 16, 16).T    # [16, n/16]
    return np.tile(w, (8, 1))


# ----------------------------------------------------------------------------
# Weight packing (shared between builder offsets and host fill)
# ----------------------------------------------------------------------------

class Pack:
    def __init__(self):
        self.cols = 0
        self.items = {}   # name -> (off, rows, ncols)

    def add(self, name, rows, ncols):
        self.items[name] = (self.cols, rows, ncols)
        self.cols += ncols

    def off(self, name):
        return self.items[name][0]


def make_packs(cfg):
    pb = Pack()           # bf16 pack [128, pb.cols]
    pb.add("ident", P, P)
    pb.add("zeros", 16, P)
    pb.add("ee_w1", cfg.EI, H)
    pb.add("ee_w2", H, H)
    pb.add("ne_w2", H, H)
    for l in range(cfg.L):
        pb.add(f"ew1a{l}", H, H)
        pb.add(f"ew1b{l}", H, H)
        pb.add(f"ew1c{l}", H, H)
        pb.add(f"ew2{l}", H, H)
        pb.add(f"nw1a{l}", H, H)
        pb.add(f"nw1b{l}", H, H)
        pb.add(f"nw2{l}", H, H)
    pb.add("nd_w2", H, cfg.NO)

    pf = Pack()           # f32 pack [128, pf.cols]
    pf.add("ne_w1", cfg.NI, H)
    pf.add("nd_w1", H, H)
    for nm in ["ne_b1", "ne_b2", "ee_b1", "ee_b2", "ndb1", "ndb2"]:
        pf.add(nm, H, 1)
    for l in range(cfg.L):
        for nm in ["eb1", "eb2", "nb1", "nb2"]:
            pf.add(f"{nm}{l}", H, 1)
    return pb, pf


def fill_packs(cfg, pb, pf, inputs):
    """Returns (wbf [128, pb.cols] bf16, wf32 [128, pf.cols] f32)."""
    W = {k: np.asarray(v, np.float32) for k, v in inputs.items()
         if k not in ("x", "edge_attr", "edge_index", "batch")}

    wbf = np.zeros((P, pb.cols), BF16)
    wf32 = np.zeros((P, pf.cols), np.float32)

    def setb(name, arr):
        off, rows, ncols = pb.items[name]
        wbf[:rows, off:off + ncols] = _bf(arr).reshape(rows, ncols)

    def setf(name, arr):
        off, rows, ncols = pf.items[name]
        wf32[:rows, off:off + ncols] = np.asarray(arr, np.float32).reshape(rows, ncols)

    setb("ident", np.eye(P))
    setb("zeros", np.zeros((16, P)))
    setb("ee_w1", W["ee_w1"])
    setb("ee_w2", W["ee_w2"])
    setb("ne_w2", W["ne_w2"])
    for l in range(cfg.L):
        setb(f"ew1a{l}", W["edge_w1"][l][:H])
        setb(f"ew1b{l}", W["edge_w1"][l][H:2 * H])
        setb(f"ew1c{l}", W["edge_w1"][l][2 * H:])
        setb(f"ew2{l}", W["edge_w2"][l])
        setb(f"nw1a{l}", W["node_w1"][l][:H])
        setb(f"nw1b{l}", W["node_w1"][l][H:])
        setb(f"nw2{l}", W["node_w2"][l])
    setb("nd_w2", W["nd_w2"])

    setf("ne_w1", W["ne_w1"])
    setf("nd_w1", W["nd_w1"])
    setf("ne_b1", W["ne_b1"]); setf("ne_b2", W["ne_b2"])
    setf("ee_b1", W["ee_b1"]); setf("ee_b2", W["ee_b2"])
    setf("ndb1", W["nd_b1"])
    ndb2 = np.zeros(P, np.float32); ndb2[:cfg.NO] = W["nd_b2"]
    setf("ndb2", ndb2)
    for l in range(cfg.L):
        setf(f"eb1{l}", W["edge_b1"][l]); setf(f"eb2{l}", W["edge_b2"][l])
        setf(f"nb1{l}", W["node_b1"][l]); setf(f"nb2{l}", W["node_b2"][l])
    return wbf, wf32


# ----------------------------------------------------------------------------
# Bass program
# ----------------------------------------------------------------------------

FLAGS = set()


def build_program(cfg, pb, pf):
    nc = bacc.Bacc("TRN2", target_bir_lowering=False,
                   debug=not axon_active(), num_devices=NCORES,
                   num_swdge_queues=4)
    L, NSH, NSH_pad, E_pad, NCH = cfg.L, cfg.NSH, cfg.NSH_pad, cfg.E_pad, cfg.NCH
    NT = NSH_pad // T           # node tiles
    NB = NSH_pad // P           # node transpose blocks
    GIDX = G // 16

    xin_d = nc.dram_tensor("xin", [cfg.NI, NSH_pad], F32, kind="ExternalInput")
    attrT_d = nc.dram_tensor("attrT", [cfg.EI, E_pad], BF, kind="ExternalInput")
    idxc_d = nc.dram_tensor("idx_col", [P, E_pad // 16], I16, kind="ExternalInput")
    ndum_d = nc.dram_tensor("ndum", [P, NSH_pad], BF, kind="ExternalInput")
    wbf_d = nc.dram_tensor("wbf", [P, pb.cols], BF, kind="ExternalInput")
    wf32_d = nc.dram_tensor("wf32", [P, pf.cols], F32, kind="ExternalInput")
    out_d = nc.dram_tensor("out", [cfg.NO, NSH], F32, kind="ExternalOutput")

    with tile.TileContext(nc) as tc:
        with (
            tc.tile_pool(name="persist", bufs=1) as pp,
            tc.tile_pool(name="work", bufs=2) as wp,
            tc.tile_pool(name="psum", bufs=1, space="PSUM") as psp,
            tc.tile_pool(name="dram", bufs=1, space="DRAM") as dp,
        ):
            # ---- persistent SBUF ----
            wbf = pp.tile([P, pb.cols], BF)
            wf32 = pp.tile([P, pf.cols], F32)
            idxc = pp.tile([P, E_pad // 16], I16)
            ndum = pp.tile([P, NSH_pad], BF)
            vdum = pp.tile([P, NSH_pad], BF)
            e_car = pp.tile([P, E_pad], BF)
            x_fp = pp.tile([P, NSH_pad], F32)
            x_bf = pp.tile([P, NSH_pad], BF)
            agg_f = pp.tile([P, NSH_pad], F32)
            agg_bf = pp.tile([P, NSH_pad], BF)
            x_rows = pp.tile([P, NB, P], BF)

            nc.sync.dma_start(out=wbf[:], in_=wbf_d[:])
            nc.sync.dma_start(out=wf32[:], in_=wf32_d[:])
            nc.sync.dma_start(out=idxc[:], in_=idxc_d[:])
            nc.sync.dma_start(out=ndum[:], in_=ndum_d[:])

            def wb(name, rows=P, cols=H):
                o = pb.off(name)
                return wbf[0:rows, o:o + cols]

            def wf(name, rows=P, cols=H):
                o = pf.off(name)
                return wf32[0:rows, o:o + cols]

            def bias(name):
                o = pf.off(name)
                return wf32[:, o:o + 1]

            ident = wb("ident")

            # ---- DRAM: per-layer x tables (Shared = single AG writer each) ----
            x_tbls = [dp.tile([cfg.NTOT, H], BF, addr_space="Shared",
                              name=f"x_tbl{li}") for li in range(L)]
            ag_in = dp.tile([cfg.STRIDE, H], BF)
            # zero row inside every core's block tail (dummy-slot gather target)
            nc.sync.dma_start(out=ag_in[NSH_pad:cfg.STRIDE, :],
                              in_=wb("zeros", rows=16, cols=P))

            nc.vector.memset(agg_f[:, NSH:NSH_pad], 0.0)

            num_reg = nc.gpsimd.to_reg(G)

            relu = mybir.ActivationFunctionType.Relu
            copyf = mybir.ActivationFunctionType.Copy
            add = mybir.AluOpType.add
            GB = G // P      # 128-row blocks per gather chunk

            def mlp_tile(dst, dst_sl, parts, b1ap, w2, b2ap, n):
                """dst[dst_sl] = (relu(sum_i w_i^T @ rhs_i + b1))^T @ w2 + b2,
                feature-major, one [*, n] tile. parts: list of (lhsT_ap, rhs_ap)."""
                h_ps = psp.tile([P, T], F32, tag="ps512", bufs=5)
                for i, (lw, rr) in enumerate(parts):
                    nc.tensor.matmul(out=h_ps[:, :n], lhsT=lw, rhs=rr,
                                     start=(i == 0), stop=(i == len(parts) - 1))
                h_sb = wp.tile([P, T], BF, tag="h_sb", bufs=4)
                nc.scalar.activation(h_sb[:, :n], h_ps[:, :n], relu, bias=b1ap)
                o_ps = psp.tile([P, T], F32, tag="ps512", bufs=5)
                nc.tensor.matmul(out=o_ps[:w2.shape[1], :n], lhsT=w2, rhs=h_sb[:, :n],
                                 start=True, stop=True)
                if cfg.zero_bias:
                    nc.scalar.activation(dst[dst_sl], o_ps[:w2.shape[1], :n], copyf)
                else:
                    nc.vector.tensor_scalar(out=dst[dst_sl], in0=o_ps[:w2.shape[1], :n],
                                            scalar1=b2ap, scalar2=None, op0=add)

            # ---- node encoder ----
            for t in range(NT):
                sl = np.s_[:, t * T:(t + 1) * T]
                xt = wp.tile([cfg.NI, T], F32, tag="attr", bufs=2)
                nc.sync.dma_start(out=xt[:], in_=xin_d[:, t * T:(t + 1) * T])
                mlp_tile(x_fp, sl, [(wf("ne_w1", rows=cfg.NI), xt[:])],
                         bias("ne_b1"), wb("ne_w2"), bias("ne_b2"), T)
                nc.vector.tensor_copy(out=x_bf[sl], in_=x_fp[sl])

            # ---- x table publish (transpose + AG) ----
            def publish_x(li):
                if "notrans" in FLAGS:
                    nc.vector.memset(x_rows[:], 0.0)
                else:
                    for j in range(NB):
                        tp = psp.tile([P, P], BF, tag="pst", bufs=2)
                        nc.tensor.transpose(out=tp[:], in_=x_bf[:, j * P:(j + 1) * P],
                                            identity=ident)
                        nc.scalar.activation(x_rows[:, j, :], tp[:], copyf)
                ag_view = ag_in[0:NSH_pad, :].rearrange("(j p) q -> p j q", p=P)
                nc.sync.dma_start(out=ag_view, in_=x_rows[:])
                nc.gpsimd.collective_compute(
                    "AllGather", mybir.AluOpType.bypass,
                    replica_groups=[list(range(NCORES))],
                    ins=[ag_in[:]], outs=[x_tbls[li][:]])

            publish_x(0)

            # ---- edge encoder ----
            GE = 2048
            for k in range(E_pad // GE):
                at = wp.tile([cfg.EI, GE], BF, tag="attr", bufs=2)
                nc.sync.dma_start(out=at[:], in_=attrT_d[:, k * GE:(k + 1) * GE])
                for t in range(GE // T):
                    s0 = k * GE + t * T
                    mlp_tile(e_car, np.s_[:, s0:s0 + T],
                             [(wb("ee_w1", rows=cfg.EI), at[:, t * T:(t + 1) * T])],
                             bias("ee_b1"), wb("ee_w2"), bias("ee_b2"), T)

            # v0: encoder output for an all-zero dummy edge (node-space mirror)
            zattr = pp.tile([cfg.EI, T], BF)
            nc.vector.memset(zattr[:], 0.0)
            for t in range(NT):
                mlp_tile(vdum, np.s_[:, t * T:(t + 1) * T],
                         [(wb("ee_w1", rows=cfg.EI), zattr[:])],
                         bias("ee_b1"), wb("ee_w2"), bias("ee_b2"), T)

            # ---- message-passing layers ----
            for l in range(L):
                for k in range(NCH):
                    xr = wp.tile([P, 1, G], BF, tag="xr", bufs=2)
                    xc = wp.tile([P, 1, G], BF, tag="xc", bufs=2)
                    if "nogather" in FLAGS:
                        nc.vector.memset(xr[:], 0.0)
                        nc.vector.memset(xc[:], 0.0)
                    else:
                        for (rel, no, n, dw) in cfg.plans[k]:
                            if no < 0:
                                nc.vector.memset(xr[:, 0, rel:rel + dw], 0.0)
                                continue
                            src = x_bf[:, no:no + n].rearrange(
                                "p (n o) -> p n o", o=1).to_broadcast([P, n, dw])
                            dst = xr[:, 0, rel:rel + n * dw].rearrange(
                                "p (n d) -> p n d", d=dw)
                            nc.scalar.activation(dst, src, copyf)
                        gp = wp.tile([P, GB, P], BF, tag="gp", bufs=2)
                        nc.gpsimd.dma_gather(gp[:], x_tbls[l][:], idxc[:, k * GIDX:(k + 1) * GIDX],
                                             G, num_reg, H, transpose=False,
                                             single_packet=False, queue_num=0)
                        for q in range(GB // 4):
                            tps = psp.tile([P, 4 * P], BF, tag="ps512", bufs=5)
                            for j in range(4):
                                nc.tensor.transpose(out=tps[:, j * P:(j + 1) * P],
                                                    in_=gp[:, 4 * q + j, :],
                                                    identity=ident)
                            nc.scalar.activation(
                                xc[:, 0, 4 * q * P:(4 * q + 4) * P], tps[:], copyf)
                    for t in range(G // T):
                        s0 = k * G + t * T
                        mlp_tile(e_car, np.s_[:, s0:s0 + T],
                                 [(wb(f"ew1a{l}"), xr[:, 0, t * T:(t + 1) * T]),
                                  (wb(f"ew1b{l}"), xc[:, 0, t * T:(t + 1) * T]),
                                  (wb(f"ew1c{l}"), e_car[:, s0:s0 + T])],
                                 bias(f"eb1{l}"), wb(f"ew2{l}"), bias(f"eb2{l}"), T)

                # dummy-value chain in node space: v = edge_mlp([x, 0, v])
                for t in range(NT):
                    sl = np.s_[:, t * T:(t + 1) * T]
                    mlp_tile(vdum, sl,
                             [(wb(f"ew1a{l}"), x_bf[sl]),
                              (wb(f"ew1c{l}"), vdum[sl])],
                             bias(f"eb1{l}"), wb(f"ew2{l}"), bias(f"eb2{l}"), T)

                # scatter-add: windowed segment reduce
                if "noreduce" in FLAGS:
                    nc.vector.memset(agg_f[:, 0:NSH], 0.0)
                else:
                    for (so, no, n, d) in cfg.groups:
                        src = e_car[:, so:so + n * d].rearrange("p (w d) -> p w d", d=d)
                        nc.vector.tensor_reduce(out=agg_f[:, no:no + n], in_=src,
                                                axis=mybir.AxisListType.X, op=add)
                # remove dummy-slot contributions: agg -= ndum * v
                for t in range(NT):
                    sl = np.s_[:, t * T:(t + 1) * T]
                    vn = wp.tile([P, T], F32, tag="attr", bufs=2)
                    nc.vector.tensor_tensor(out=vn[:], in0=vdum[sl], in1=ndum[sl],
                                            op=mybir.AluOpType.mult)
                    nc.vector.tensor_tensor(out=agg_f[sl], in0=agg_f[sl], in1=vn[:],
                                            op=mybir.AluOpType.subtract)
                nc.vector.tensor_copy(out=agg_bf[:], in_=agg_f[:])

                # node MLP + residual
                for t in range(NT):
                    sl = np.s_[:, t * T:(t + 1) * T]
                    h_ps = psp.tile([P, T], F32, tag="ps512", bufs=5)
                    nc.tensor.matmul(out=h_ps[:], lhsT=wb(f"nw1a{l}"), rhs=x_bf[sl],
                                     start=True, stop=False)
                    nc.tensor.matmul(out=h_ps[:], lhsT=wb(f"nw1b{l}"), rhs=agg_bf[sl],
                                     start=False, stop=True)
                    h_sb = wp.tile([P, T], BF, tag="h_sb", bufs=4)
                    nc.scalar.activation(h_sb[:], h_ps[:], relu, bias=bias(f"nb1{l}"))
                    o_ps = psp.tile([P, T], F32, tag="ps512", bufs=5)
                    nc.tensor.matmul(out=o_ps[:], lhsT=wb(f"nw2{l}"), rhs=h_sb[:],
                                     start=True, stop=True)
                    # x_fp = (o_ps + nb2) + x_fp  (residual, fp32)
                    if "nostt" in FLAGS:
                        nc.vector.tensor_copy(out=x_fp[sl], in_=o_ps[:])
                    elif cfg.zero_bias:
                        nc.vector.tensor_tensor(out=x_fp[sl], in0=o_ps[:], in1=x_fp[sl],
                                                op=add)
                    else:
                        nc.vector.scalar_tensor_tensor(out=x_fp[sl], in0=o_ps[:],
                                                       scalar=bias(f"nb2{l}"), in1=x_fp[sl],
                                                       op0=add, op1=add)
                    nc.vector.tensor_copy(out=x_bf[sl], in_=x_fp[sl])

                if l < L - 1:
                    publish_x(l + 1)

            # ---- decoder ----
            for t in range(NT):
                sl = np.s_[:, t * T:(t + 1) * T]
                h_ps = psp.tile([P, T], F32, tag="ps512", bufs=5)
                nc.tensor.matmul(out=h_ps[:], lhsT=wf("nd_w1"), rhs=x_fp[sl],
                                 start=True, stop=True)
                h_sb = wp.tile([P, T], BF, tag="h_sb", bufs=4)
                nc.scalar.activation(h_sb[:], h_ps[:], relu, bias=bias("ndb1"))
                o_ps = psp.tile([cfg.NO, T], F32, tag="ps512", bufs=5)
                nc.tensor.matmul(out=o_ps[:], lhsT=wb("nd_w2", cols=cfg.NO), rhs=h_sb[:],
                                 start=True, stop=True)
                ot = wp.tile([cfg.NO, T], F32, tag="attr", bufs=2)
                if cfg.zero_bias:
                    nc.scalar.activation(ot[:], o_ps[:], copyf)
                else:
                    nc.vector.tensor_scalar(out=ot[:], in0=o_ps[:],
                                            scalar1=bias("ndb2")[0:cfg.NO, :],
                                            scalar2=None, op0=add)
                n_real = min(T, NSH - t * T)
                if n_real > 0:
                    nc.sync.dma_start(out=out_d[:, t * T:t * T + n_real],
                                      in_=ot[:, :n_real])

    nc.compile()
    return nc


# ----------------------------------------------------------------------------
# Driver
# ----------------------------------------------------------------------------

def make_in_maps(cfg, pb, pf, inputs, per_core):
    wbf, wf32 = fill_packs(cfg, pb, pf, inputs)
    in_maps = []
    for c in range(NCORES):
        pc = per_core[c]
        in_maps.append({
            "xin": pc["xin"],
            "attrT": _bf(pc["attrT"]),
            "ndum": np.tile(_bf(pc["ndum"])[None, :], (P, 1)),
            "idx_col": _wrap_idx(pc["col_idx"]),
            "wbf": wbf,
            "wf32": wf32,
        })
    return in_maps


def assemble_output(cfg, per_core, results):
    out = np.zeros((cfg.N, cfg.NO), np.float32)
    for c in range(NCORES):
        o = results[c]["out"]                      # [NO, NSH]
        out[per_core[c]["nodes_c"]] = o.T
    return out


_cache = {}


def kernel(**inputs) -> np.ndarray:
    cfg, per_core = preprocess(inputs)
    pb, pf = make_packs(cfg)
    key = (cfg.N, cfg.E, cfg.L, cfg.E_pad, tuple(cfg.groups), cfg.zero_bias)
    if key not in _cache:
        _cache[key] = build_program(cfg, pb, pf)
    nc = _cache[key]
    in_maps = make_in_maps(cfg, pb, pf, inputs, per_core)
    res = run_bass_kernel_spmd(nc, in_maps, list(range(NCORES)))
    return assemble_output(cfg, per_core, res.results)


if __name__ == "__main__":
    # quick self-drive with random mini inputs
    rng = np.random.default_rng(0)
    N, E, L, NI, EI, NO = 1024, 8192, 2, 6, 3, 3
    Hd = 128
    inputs = dict(
        x=rng.standard_normal((N, NI)).astype(np.float32),
        edge_attr=rng.standard_normal((E, EI)).astype(np.float32),
        edge_index=rng.integers(0, N, (2, E)).astype(np.int32),
        batch=np.zeros(N, np.int32),
        ne_w1=rng.standard_normal((NI, Hd)).astype(np.float32) / np.sqrt(NI),
        ne_b1=np.zeros(Hd, np.float32),
        ne_w2=rng.standard_normal((Hd, Hd)).astype(np.float32) / np.sqrt(Hd),
        ne_b2=np.zeros(Hd, np.float32),
        ee_w1=rng.standard_normal((EI, Hd)).astype(np.float32) / np.sqrt(EI),
        ee_b1=np.zeros(Hd, np.float32),
        ee_w2=rng.standard_normal((Hd, Hd)).astype(np.float32) / np.sqrt(Hd),
        ee_b2=np.zeros(Hd, np.float32),
        edge_w1=rng.standard_normal((L, 3 * Hd, Hd)).astype(np.float32) / np.sqrt(3 * Hd),
        edge_b1=np.zeros((L, Hd), np.float32),
        edge_w2=rng.standard_normal((L, Hd, Hd)).astype(np.float32) / np.sqrt(Hd),
        edge_b2=np.zeros((L, Hd), np.float32),
        node_w1=rng.standard_normal((L, 2 * Hd, Hd)).astype(np.float32) / np.sqrt(2 * Hd),
        node_b1=np.zeros((L, Hd), np.float32),
        node_w2=rng.standard_normal((L, Hd, Hd)).astype(np.float32) / np.sqrt(Hd),
        node_b2=np.zeros((L, Hd), np.float32),
        nd_w1=rng.standard_normal((Hd, Hd)).astype(np.float32) / np.sqrt(Hd),
        nd_b1=np.zeros(Hd, np.float32),
        nd_w2=rng.standard_normal((Hd, NO)).astype(np.float32) / np.sqrt(Hd),
        nd_b2=np.zeros(NO, np.float32),
    )
    got = kernel(**inputs)

    # numpy reference
    def mlp2(h, w1, b1, w2, b2):
        return np.maximum(h @ w1 + b1, 0.0) @ w2 + b2
    xx = mlp2(inputs["x"], inputs["ne_w1"], inputs["ne_b1"], inputs["ne_w2"], inputs["ne_b2"])
    e = mlp2(inputs["edge_attr"], inputs["ee_w1"], inputs["ee_b1"], inputs["ee_w2"], inputs["ee_b2"])
    r, cl = inputs["edge_index"][0], inputs["edge_index"][1]
    for l in range(L):
        msg = np.concatenate([xx[r], xx[cl], e], 1)
        e_new = mlp2(msg, inputs["edge_w1"][l], inputs["edge_b1"][l],
                     inputs["edge_w2"][l], inputs["edge_b2"][l])
        agg = np.zeros_like(xx)
        np.add.at(agg, r, e_new)
        x_new = mlp2(np.concatenate([xx, agg], 1), inputs["node_w1"][l],
                     inputs["node_b1"][l], inputs["node_w2"][l], inputs["node_b2"][l])
        xx, e = x_new + xx, e_new
    want = mlp2(xx, inputs["nd_w1"], inputs["nd_b1"], inputs["nd_w2"], inputs["nd_b2"])
    err = np.linalg.norm(got - want) / np.linalg.norm(want)
    print("mini rel l2 err:", err)
    print("max abs err:", np.abs(got - want).max(), "scale:", np.abs(want).max())
